# revision 70
# baseline (speedup 1.0000x reference)
"""Trainium2 Bass kernel v3 for nn_G_HGNN_layer_38448547234609.

HGNN layer: knn-hypergraph (top-11 of 8192 merged nodes) + static local
window hyperedges, G = Dv^-1/2 H De^-1 H^T Dv^-1/2, linear -> G@y ->
batchnorm(train) -> relu -> residual.  Never materializes G:
z = dv2 * (Hfull @ ((1/DE) * (Hfull^T @ (dv2*y)))).

v4 design (one sample per core, 8 row-tiles of 128):
 - distances via 3-term fp16 hi/lo split matmuls (exact top-11 on this
   input; fp32 d in SBUF for exact threshold compare — fewer passes or
   16-bit d storage provably corrupt the topology: even 2 wrong mask
   elements push the output past the tolerance, BN amplifies ~25x).
 - per-row exact 11th/12th-largest via chunk top-8 (DVE Max) + one
   match_replace round; mask = is_ge(d, v11) split across DVE/POOL; the
   last tile finishes 3-way (ACT sigmoid(1e6*(d - midpoint)) + DVE +
   POOL) to release the phase-A pool barrier ~3us sooner.
 - tiles 4..7 masks stay SBUF-resident; 0..3 spill to DRAM and are
   re-streamed per u region (the ms tiles stay live so consume() can
   PE-transpose them — no DRAM re-read for the z side).
 - u^T in PSUM [65, 1024] regions (8 of them), drained bf16, AllReduce
   issued via gpsimd SWDGE (bypasses the serial HWDGE device, whose
   round-robin counting semaphores otherwise serialize the pipeline);
   produce/consume interleaved one region apart to hide the collective.
 - consume(r): maskT chunks [128, 1024] assembled from 9 PE transposes
   (4 streamed + 4 resident + ur chunk) with the dv2_i column scale
   folded into the DVE assembly drains (local hlocT is dv2-scaled on
   the host), so z^T needs no separate dv2 pass.
 - BN stats: DVE row-sum + ACT Square-accum read z^T straight from
   PSUM; tiny AllReduce; scale/relu/residual in transposed layout
   (single Sqrt act-table set pinned at head); host transposes back.
"""

import numpy as np
import ml_dtypes

import concourse.bass as bass
import concourse.bacc as bacc
import concourse.mybir as mybir
import concourse.tile as tile
from concourse import bass_utils

AF = mybir.ActivationFunctionType
ALU = mybir.AluOpType
F32 = mybir.dt.float32
F16 = mybir.dt.float16
BF16 = mybir.dt.bfloat16

NODE, K, KER, STR = 32, 10, 5, 2
B, C = 8, 64
N = NODE * NODE            # 1024 nodes/sample
BN = B * N                 # 8192
OUT_ = (NODE - KER) // STR + 1
E = OUT_ * OUT_            # 196 local hyperedges/sample
NCORE = 8
NT = 8                     # 128-row tiles per core
BN_EPS = 1e-5
BIG = 1e30
RW = 1024                  # u^T region width (2 PSUM banks)
NREG = BN // RW            # 8 regions
NCH = RW // 128            # 16 maskT chunks per region
NRES = 4                   # mask tiles kept resident in SBUF (tiles 4..7)
SGS = 1.0e6                # sigmoid sharpness for the ACT mask share


_CACHE = {}
SIM_NO_CC = False  # replace collectives with DMA copies (for TimelineSim)


def _local_incidence():
    idx = np.arange(N).reshape(NODE, NODE)
    H_local = np.zeros((N, E), np.float32)
    e = 0
    for i in range(0, NODE - KER + 1, STR):
        for j in range(0, NODE - KER + 1, STR):
            H_local[idx[i:i + KER, j:j + KER].ravel(), e] = 1.0
            e += 1
    return H_local


def _build():
    nc = bacc.Bacc(num_devices=NCORE)

    bzh = nc.dram_tensor("bzh", [65, BN], F16, kind="ExternalInput")
    bzl = nc.dram_tensor("bzl", [65, BN], F16, kind="ExternalInput")
    ac = nc.dram_tensor("ac", [65, 2 * N], F16, kind="ExternalInput")
    wbh = nc.dram_tensor("wbh", [65, C], F16, kind="ExternalInput")
    wbl = nc.dram_tensor("wbl", [65, C], F16, kind="ExternalInput")
    dv2t = nc.dram_tensor("dv2t", [128, NT], F32, kind="ExternalInput")
    dv2row = nc.dram_tensor("dv2row", [1, N], BF16, kind="ExternalInput")
    hloc = nc.dram_tensor("hloc", [128, NT * E], BF16, kind="ExternalInput")
    hlocT = nc.dram_tensor("hlocT", [98, 2 * N], BF16, kind="ExternalInput")
    identb = nc.dram_tensor("identb", [128, 128], BF16, kind="ExternalInput")
    gammaP = nc.dram_tensor("gammaP", [C, 1], F32, kind="ExternalInput")
    betaP = nc.dram_tensor("betaP", [C, 1], F32, kind="ExternalInput")
    xTres = nc.dram_tensor("xTres", [C, N], F32, kind="ExternalInput")
    out = nc.dram_tensor("out", [C, N], F32, kind="ExternalOutput")

    with tile.TileContext(nc) as tc:
        with (
            tc.tile_pool(name="const", bufs=1) as cp,
            tc.tile_pool(name="persist", bufs=1) as pp,
            tc.tile_pool(name="small", bufs=4) as sp,
            tc.tile_pool(name="dram", bufs=1, space="DRAM") as dr,
            tc.tile_pool(name="mres", bufs=1) as mrp,
            tc.tile_pool(name="pu", bufs=1, space="PSUM") as pup,
        ):
            # ---- whole-program constants (loaded after the bz stream) ----
            dv2_sb = cp.tile([128, NT], F32, tag="dv2")
            wbh_sb = cp.tile([65, C], F16, tag="wbh")
            wbl_sb = cp.tile([65, C], F16, tag="wbl")

            u_tiles = {}

            def get_u(r):
                if r not in u_tiles:
                    u_tiles[r] = pup.tile([65, RW], F32, tag="u", name=f"u_ps{r}")
                return u_tiles[r]

            aseed = sp.tile([1, 1], F32, tag="aseed")
            nc.vector.memset(aseed[:], 1.0)
            nc.scalar.activation(aseed[:], aseed[:], AF.Sqrt, bias=0.0, scale=1.0)
            m_aug = pp.tile([128, NT * 65], BF16, tag="maug")
            vloc_sb = pp.tile([98, 2 * C], BF16, tag="vloc")

            # tiles 4..7 stay resident in SBUF; 0..3 are re-streamed for u
            mask_res = [mrp.tile([128, BN], BF16, tag=f"mres{t}",
                                 name=f"mask_res{t}") for t in range(NRES)]
            maskD = dr.tile([(NT - NRES) * 128, BN], BF16, tag="maskD",
                            name="maskD")
            cc_in = [dr.tile([65, RW], BF16, tag=f"ccin{r}", name=f"cc_in{r}")
                     for r in range(NREG)]
            cc_out = [dr.tile([65, RW], BF16, tag=f"ccout{r}", name=f"cc_out{r}",
                              addr_space="Shared")
                      for r in range(NREG)]

            # ================= P0 + A: features, distances, masks =============
            with tc.tile_pool(name="early", bufs=1) as ep:
                ac_sb = cp.tile([65, 2 * N], F16, tag="ac")
                nc.sync.dma_start(ac_sb[:], ac[:])
                bzh_sb = ep.tile([65, BN], F16, tag="bzh")
                bzl_sb = ep.tile([65, BN], F16, tag="bzl")
                for q in range(4):
                    q0, q1 = q * (BN // 4), (q + 1) * (BN // 4)
                    nc.sync.dma_start(bzh_sb[:, q0:q1], bzh[:, q0:q1])
                    nc.sync.dma_start(bzl_sb[:, q0:q1], bzl[:, q0:q1])
                nc.sync.dma_start(wbh_sb[:], wbh[:])
                nc.sync.dma_start(wbl_sb[:], wbl[:])
                nc.sync.dma_start(dv2_sb[:], dv2t[:])
                hloc_sb = cp.tile([128, NT * E], BF16, tag="hloc")
                nc.sync.dma_start(hloc_sb[:], hloc[:])
                idb_sb = cp.tile([128, 128], BF16, tag="identb")
                nc.sync.dma_start(idb_sb[:], identb[:])
                gam_sb = cp.tile([C, 1], F32, tag="gammaP")
                nc.sync.dma_start(gam_sb[:], gammaP[:])
                bet_sb = cp.tile([C, 1], F32, tag="betaP")
                nc.sync.dma_start(bet_sb[:], betaP[:])

                # ---- A: distance row-tiles, exact top-11 threshold, masks ----
                with (
                    tc.tile_pool(name="dwork", bufs=4) as dwp,
                    tc.tile_pool(name="mwork", bufs=2) as mwp,
                    tc.tile_pool(name="pd", bufs=3, space="PSUM") as pdp,
                ):
                    for t in range(NT):
                        a_h = ac_sb[:, t * 128:(t + 1) * 128]
                        a_l = ac_sb[:, N + t * 128:N + (t + 1) * 128]
                        d_half = [dwp.tile([128, BN // 2], F32, tag="dsb",
                                           name=f"d_half{t}_{hh}")
                                  for hh in range(2)]
                        cand = sp.tile([128, 64], F32, tag="cand")
                        for k in range(8):
                            d_sb = d_half[k // 4]
                            kb0 = (k % 4) * 1024
                            d_ps = pdp.tile([128, 1024], F32, tag="dch")
                            for h in range(2):
                                kk = 2 * k + h
                                b_h = bzh_sb[:, kk * 512:(kk + 1) * 512]
                                b_l = bzl_sb[:, kk * 512:(kk + 1) * 512]
                                dsl = d_ps[:, h * 512:(h + 1) * 512]
                                nc.tensor.matmul(dsl, lhsT=a_h, rhs=b_h,
                                                 start=True, stop=False)
                                nc.tensor.matmul(dsl, lhsT=a_h, rhs=b_l,
                                                 start=False, stop=False)
                                nc.tensor.matmul(dsl, lhsT=a_l, rhs=b_h,
                                                 start=False, stop=True)
                            nc.scalar.copy(d_sb[:, kb0:kb0 + 1024], d_ps[:])
                            nc.vector.max(cand[:, k * 8:(k + 1) * 8],
                                          d_sb[:, kb0:kb0 + 1024])
                        # exact 11th/12th largest via candidates
                        c8a = sp.tile([128, 8], F32, tag="c8")
                        nc.vector.max(c8a[:], cand[:])
                        nc.vector.match_replace(cand[:], c8a[:], cand[:], -BIG)
                        c8b = sp.tile([128, 8], F32, tag="c8")
                        nc.vector.max(c8b[:], cand[:])
                        sgb = sp.tile([128, 1], F32, tag="sgb")
                        nc.vector.tensor_tensor(sgb[:], c8b[:, 2:3],
                                                c8b[:, 3:4], ALU.add)
                        nc.vector.tensor_scalar(sgb[:], sgb[:], -SGS * 0.5,
                                                None, ALU.mult)
                        if t >= NT - NRES:
                            mk = mask_res[t - (NT - NRES)]
                        else:
                            mk = mwp.tile([128, BN], BF16, tag="mk")
                        if t == NT - 1:
                            # last tile gates the pool-close barrier: finish
                            # the mask asap with a 3-way ACT/DVE/POOL split;
                            # ACT's share is sigmoid(SGS*(d - midpoint)),
                            # exact as long as v11 > v12 in fp32 (verified)
                            nc.scalar.activation(
                                mk[:, 0:3900], d_half[0][:, 0:3900], AF.Sigmoid,
                                bias=sgb[:, 0:1], scale=SGS)
                            nc.vector.tensor_scalar(
                                mk[:, 3900:4096], d_half[0][:, 3900:4096],
                                c8b[:, 2:3], None, ALU.is_ge)
                            nc.vector.tensor_scalar(
                                mk[:, 4096:7000], d_half[1][:, 0:2904],
                                c8b[:, 2:3], None, ALU.is_ge)
                            nc.gpsimd.tensor_scalar(
                                mk[:, 7000:BN], d_half[1][:, 2904:BN // 2],
                                c8b[:, 2:3], None, ALU.is_ge)
                        else:
                            for hh in range(2):
                                j0 = hh * (BN // 2)
                                nc.vector.tensor_scalar(
                                    mk[:, j0:j0 + 514], d_half[hh][:, 0:514],
                                    c8b[:, 2:3], None, ALU.is_ge)
                                nc.gpsimd.tensor_scalar(
                                    mk[:, j0 + 514:j0 + BN // 2],
                                    d_half[hh][:, 514:BN // 2],
                                    c8b[:, 2:3], None, ALU.is_ge)
                        # spill streamed tiles (u reload source in phase 3)
                        if t < NT - NRES:
                            nc.sync.dma_start(
                                maskD[t * 128:(t + 1) * 128, :], mk[:])

            # ---- P0 (deferred): y = x W^T + b; m_aug = bf16(dv2*y)|1 ----
            with tc.tile_pool(name="py", bufs=2, space="PSUM") as pyp:
                for t in range(NT):
                    a_h = ac_sb[:, t * 128:(t + 1) * 128]
                    a_l = ac_sb[:, N + t * 128:N + (t + 1) * 128]
                    y_ps = pyp.tile([128, C], F32, tag="y")
                    nc.tensor.matmul(y_ps[:], lhsT=a_h, rhs=wbh_sb[:],
                                     start=True, stop=False)
                    nc.tensor.matmul(y_ps[:], lhsT=a_h, rhs=wbl_sb[:],
                                     start=False, stop=False)
                    nc.tensor.matmul(y_ps[:], lhsT=a_l, rhs=wbh_sb[:],
                                     start=False, stop=True)
                    nc.scalar.activation(m_aug[:, t * 65:t * 65 + C], y_ps[:],
                                         AF.Copy, bias=0.0,
                                         scale=dv2_sb[:, t:t + 1])
                    nc.vector.memset(m_aug[:, t * 65 + C:t * 65 + 65], 1.0)

            # ---- local totals tl = hloc^T m (deferred: off the head path) ----
            with tc.tile_pool(name="ptl", bufs=1, space="PSUM") as ptlp:
                tl_ps = [ptlp.tile([98, C], F32, tag=f"tl{ec}", name=f"tl_ps{ec}")
                         for ec in range(2)]
                for t in range(NT):
                    for ec in range(2):
                        nc.tensor.matmul(
                            tl_ps[ec][:],
                            lhsT=hloc_sb[:, t * E + ec * 98:t * E + ec * 98 + 98],
                            rhs=m_aug[:, t * 65:t * 65 + C],
                            start=(t == 0), stop=(t == NT - 1))
                for ec in range(2):
                    nc.scalar.activation(vloc_sb[:, ec * C:(ec + 1) * C],
                                         tl_ps[ec][:], AF.Copy, bias=0.0,
                                         scale=1.0 / 25.0)

            # ========== fused pipeline: u^T regions + CC + v + z^T ============
            lp_cm = tc.tile_pool(name="late", bufs=1)
            lp = lp_cm.__enter__()

            def late_loads():
                xt_sb = lp.tile([C, N], F32, tag="xTres")
                nc.scalar.dma_start(xt_sb[:], xTres[:])
                hlocT_sb = lp.tile([98, 2 * N], BF16, tag="hlocT")
                nc.scalar.dma_start(hlocT_sb[:], hlocT[:])
                dv2r_sb = lp.tile([1, N], BF16, tag="dv2r")
                nc.scalar.dma_start(dv2r_sb[:], dv2row[:])
                dv2b = lp.tile([128, N], BF16, tag="dv2b")
                nc.gpsimd.partition_broadcast(dv2b[:], dv2r_sb[:])
                return xt_sb, hlocT_sb, dv2b

            with (
                tc.tile_pool(name="mstream", bufs=16) as msp,
                tc.tile_pool(name="ucc", bufs=3) as uccp,
                tc.tile_pool(name="urx", bufs=3) as urxp,
                tc.tile_pool(name="vp", bufs=1) as vp,
                tc.tile_pool(name="mtp", bufs=6) as mtp,
                tc.tile_pool(name="pz", bufs=1, space="PSUM") as pzp,
                tc.tile_pool(name="pmt", bufs=2, space="PSUM") as pmtp,
            ):
                zT_ps = pzp.tile([64, N], F32, tag="zT")
                v_sb = vp.tile([128, (BN // 128) * C], BF16, tag="vsb")
                ms_live = {}   # (r, t) -> ms tile streamed in produce(r)

                def local_z():
                    # local part of z^T: zT = sum_ec vloc_ec-weights x hlocT_ec
                    for ec in range(2):
                        for kb in range(2):
                            nc.tensor.matmul(
                                zT_ps[:, kb * 512:(kb + 1) * 512],
                                lhsT=vloc_sb[:, ec * C:(ec + 1) * C],
                                rhs=hlocT_sb[:, ec * N + kb * 512:
                                             ec * N + (kb + 1) * 512],
                                start=(ec == 0), stop=False,
                                skip_group_check=True)

                def produce(r):
                    """u^T matmuls for region r, drain bf16, SWDGE AllReduce.
                    Tiles 0..3 stream from maskD (ACT-issued loads); the ms
                    tiles stay live so consume(r) can PE-transpose them.
                    Regions 0-1 already accumulated tiles 4..6 in phase A."""
                    u_ps = get_u(r)
                    for t in range(4):
                        ms = msp.tile([128, RW], BF16, tag="ms")
                        nc.scalar.dma_start(
                            ms[:], maskD[t * 128:(t + 1) * 128,
                                         r * RW:(r + 1) * RW])
                        ms_live[(r, t)] = ms
                    order = [4, 5, 6, 0, 1, 2, 3, 7]
                    for ti, t in enumerate(order):
                        if t >= NT - NRES:
                            rhs_tile, off = mask_res[t - (NT - NRES)], r * RW
                        else:
                            rhs_tile, off = ms_live[(r, t)], 0
                        for kb in range(RW // 512):
                            nc.tensor.matmul(
                                u_ps[:, kb * 512:(kb + 1) * 512],
                                lhsT=m_aug[:, t * 65:(t + 1) * 65],
                                rhs=rhs_tile[:, off + kb * 512:off + (kb + 1) * 512],
                                start=(ti == 0),
                                stop=(ti == len(order) - 1),
                                skip_group_check=True)
                    ucc = uccp.tile([65, RW], BF16, tag="ucc")
                    nc.scalar.copy(ucc[:, 0:RW // 2], u_ps[:, 0:RW // 2])
                    nc.scalar.copy(ucc[:, RW // 2:RW], u_ps[:, RW // 2:RW])
                    nc.gpsimd.dma_start(cc_in[r][:], ucc[:])
                    if SIM_NO_CC:
                        nc.gpsimd.dma_start(cc_out[r][:], cc_in[r][:])
                    else:
                        nc.gpsimd.collective_compute(
                            "AllReduce", ALU.add,
                            replica_groups=[list(range(NCORE))],
                            ins=[cc_in[r].opt()], outs=[cc_out[r].opt()])

                pend = []   # (jc, mkt) assembled, z-matmuls not yet issued

                def z_mm(last):
                    jc, mkt = pend.pop(0)
                    for kb in range(2):
                        nc.tensor.matmul(
                            zT_ps[:, kb * 512:(kb + 1) * 512],
                            lhsT=v_sb[:, jc * C:(jc + 1) * C],
                            rhs=mkt[:, kb * 512:(kb + 1) * 512],
                            start=False, stop=(last and kb == 1),
                            skip_group_check=True)

                def consume(r, last):
                    """maskT chunks fully PE-transposed (residents + region ms
                    tiles) + v + z^T accumulation, region r."""
                    urx = urxp.tile([65, RW], BF16, tag="urt")
                    nc.gpsimd.dma_start(urx[:], cc_out[r][:])
                    for jcl in range(NCH):
                        jc = r * NCH + jcl
                        mkt = mtp.tile([128, N], BF16, tag="mkt")
                        # PE: streamed tiles 0..3 + ur chunk into psum A
                        mt_a = pmtp.tile([128, 577], BF16, tag="mtpa")
                        for t in range(4):
                            nc.tensor.transpose(
                                mt_a[:, t * 128:(t + 1) * 128],
                                ms_live[(r, t)][:, jcl * 128:(jcl + 1) * 128],
                                idb_sb[:])
                        nc.tensor.transpose(mt_a[:, 512:577],
                                            urx[:, jcl * 128:(jcl + 1) * 128],
                                            idb_sb[0:65, 0:65])
                        # PE: resident tiles 4..7 into psum B
                        mt_b = pmtp.tile([128, 512], BF16, tag="mtpb")
                        for tr in range(NRES):
                            nc.tensor.transpose(
                                mt_b[:, tr * 128:(tr + 1) * 128],
                                mask_res[tr][:, jc * 128:(jc + 1) * 128],
                                idb_sb[:])
                        rec = sp.tile([128, 1], F32, tag="rec")
                        nc.vector.reciprocal(rec[:], mt_a[:, 576:577])
                        nc.vector.tensor_scalar(v_sb[:, jc * C:(jc + 1) * C],
                                                mt_a[:, 512:576],
                                                rec[:, 0:1], None, ALU.mult)
                        # assemble maskT chunk scaled by dv2_i (both DVE 2x)
                        nc.vector.tensor_tensor(
                            mkt[:, 0:512], mt_a[:, 0:512],
                            dv2b[:, 0:512], ALU.mult)
                        nc.vector.tensor_tensor(
                            mkt[:, 512:1024], mt_b[:],
                            dv2b[:, 512:1024], ALU.mult)
                        pend.append((jc, mkt))
                        if len(pend) > 1:
                            z_mm(False)

                produce(0)
                xt_sb, hlocT_sb, dv2b = late_loads()
                produce(1)
                local_z()
                for r in range(2, NREG):
                    produce(r)
                    consume(r - 2, False)
                consume(NREG - 2, False)
                consume(NREG - 1, True)
                z_mm(True)

                # ========== F: BN stats, apply, residual (zT in PSUM) =====
                st_sb = sp.tile([64, 2], F32, tag="st")
                nc.vector.tensor_reduce(st_sb[:, 0:1], zT_ps[:],
                                        axis=mybir.AxisListType.X, op=ALU.add)
                sqj = lp.tile([64, N], F32, tag="sqj")
                nc.scalar.activation(sqj[:], zT_ps[:], AF.Square, bias=0.0,
                                     scale=1.0, accum_out=st_sb[:, 1:2])
                st_in = dr.tile([64, 2], F32, tag="stin")
                st_out = dr.tile([64, 2], F32, tag="stout", addr_space="Shared")
                nc.sync.dma_start(st_in[:], st_sb[:])
                if SIM_NO_CC:
                    nc.sync.dma_start(st_out[:], st_in[:])
                else:
                    nc.gpsimd.collective_compute(
                        "AllReduce", ALU.add,
                        replica_groups=[list(range(NCORE))],
                        ins=[st_in.opt()], outs=[st_out.opt()])
                stg = sp.tile([64, 2], F32, tag="stg")
                nc.scalar.dma_start(stg[:], st_out[:])

                m2 = sp.tile([64, 2], F32, tag="m2")
                nc.vector.tensor_scalar(m2[:], stg[:], 1.0 / BN, None, ALU.mult)
                mu = m2[:, 0:1]
                negvar = sp.tile([64, 1], F32, tag="negvar")
                nc.vector.scalar_tensor_tensor(negvar[:], m2[:, 0:1], m2[:, 0:1],
                                               m2[:, 1:2], ALU.mult, ALU.subtract)
                eps_t = sp.tile([64, 1], F32, tag="eps")
                nc.vector.memset(eps_t[:], BN_EPS)
                sd = sp.tile([64, 1], F32, tag="sd")
                nc.scalar.activation(sd[:], negvar[:], AF.Sqrt, bias=eps_t[:, 0:1],
                                     scale=-1.0)
                inv = sp.tile([64, 1], F32, tag="inv")
                nc.vector.reciprocal(inv[:], sd[:])
                srow = sp.tile([64, 1], F32, tag="srow")
                nc.vector.tensor_tensor(srow[:], gam_sb[:], inv[:], ALU.mult)
                msr = sp.tile([64, 1], F32, tag="msr")
                nc.vector.tensor_tensor(msr[:], mu, srow[:], ALU.mult)
                trow = sp.tile([64, 1], F32, tag="trow")
                nc.vector.tensor_tensor(trow[:], bet_sb[:], msr[:], ALU.subtract)

                outT = lp.tile([64, N], F32, tag="outT")
                for hh in range(2):
                    c0, c1 = hh * (N // 2), (hh + 1) * (N // 2)
                    nc.scalar.activation(outT[:, c0:c1], zT_ps[:, c0:c1], AF.Relu,
                                         bias=trow[:, 0:1], scale=srow[:, 0:1])
                    nc.vector.tensor_tensor(outT[:, c0:c1], outT[:, c0:c1],
                                            xt_sb[:, c0:c1], ALU.add)
                    nc.sync.dma_start(out[:, c0:c1], outT[:, c0:c1])
            lp_cm.__exit__(None, None, None)

    nc.compile()
    return nc


def _split_f16(a):
    hi = a.astype(np.float16)
    lo = (a - hi.astype(np.float32)).astype(np.float16)
    return np.ascontiguousarray(hi), np.ascontiguousarray(lo)


def _host_inputs(x, W_conv, b_conv, gamma, beta):
    bf = ml_dtypes.bfloat16
    xm = np.ascontiguousarray(x.reshape(BN, C).astype(np.float32))
    xT = np.ascontiguousarray(xm.T)
    sq = (xm * xm).sum(1).astype(np.float32)

    bz = np.concatenate([2.0 * xT, -sq[None, :]], 0).astype(np.float32)
    bzh, bzl = _split_f16(bz)
    wbm = np.concatenate([W_conv.T.astype(np.float32),
                          b_conv[None, :].astype(np.float32)], 0)
    wbh, wbl = _split_f16(wbm)

    H_local = _local_incidence()
    cover = H_local.sum(1)
    dv2 = ((K + 1 + cover) ** -0.5).astype(np.float32)
    dv2t = dv2.reshape(NT, 128).T.copy()          # [128, NT]
    dv2row = np.ascontiguousarray(dv2[None, :])   # [1, N]

    hloc = np.zeros((128, NT * E), np.float32)
    for t in range(NT):
        hloc[:, t * E:(t + 1) * E] = H_local[t * 128:(t + 1) * 128, :]
    hlocT = np.zeros((98, 2 * N), np.float32)
    for ec in range(2):
        hlocT[:, ec * N:(ec + 1) * N] = (H_local[:, ec * 98:ec * 98 + 98]
                                         * dv2[:, None]).T

    common = {
        "bzh": bzh, "bzl": bzl,
        "wbh": wbh, "wbl": wbl,
        "dv2t": dv2t, "dv2row": dv2row.astype(bf),
        "hloc": hloc.astype(bf),
        "hlocT": hlocT.astype(bf),
        "identb": np.eye(128, dtype=np.float32).astype(ml_dtypes.bfloat16),
        "gammaP": np.ascontiguousarray(gamma.astype(np.float32)[:, None]),
        "betaP": np.ascontiguousarray(beta.astype(np.float32)[:, None]),
    }
    in_maps = []
    for c in range(NCORE):
        ac = np.concatenate([xT[:, c * N:(c + 1) * N], np.ones((1, N), np.float32)], 0)
        ach, acl = _split_f16(ac)
        m = dict(common)
        m["ac"] = np.ascontiguousarray(np.concatenate([ach, acl], axis=1))
        m["xTres"] = np.ascontiguousarray(xT[:, c * N:(c + 1) * N])
        in_maps.append(m)
    return in_maps


def _get_nc():
    if "nc" not in _CACHE:
        _CACHE["nc"] = _build()
    return _CACHE["nc"]


def run_spmd(inputs, **kw):
    nc = _get_nc()
    in_maps = _host_inputs(inputs["x"], inputs["W_conv"], inputs["b_conv"],
                           inputs["gamma"], inputs["beta"])
    return bass_utils.run_bass_kernel_spmd(nc, in_maps, core_ids=list(range(NCORE)), **kw)


def kernel(**inputs):
    res = run_spmd(inputs)
    outT = np.stack([res.results[c]["out"] for c in range(NCORE)], 0)  # [8, C, N]
    return np.ascontiguousarray(outT.transpose(0, 2, 1)).reshape(B, N, C).astype(np.float32)
